# revision 28
# baseline (speedup 1.0000x reference)
"""LoRALinear fused kernel for 8 trn2 NeuronCores — v3.

y = x @ (base + 2*(B@A))^T + bias,  x:[2,2048,4096], base:[4096,4096],
A:[8,4096], B:[4096,8], bias:[4096] -> y:[2,2048,4096], all fp32.

Sharding: 8-way token-parallel (data-parallel, replicated weights).
Per core: y_c[512, 4096] = x_c[512,4096] @ W[4096,4096]^T + bias,
decomposed as
  y_c = x_c@base^T + [x_c@A^T | 1] @ [2*B^T ; bias].

All matmul operands are fp16 (PSUM accumulation fp32). For f32r every
InstMatmult self-loads its stationary and the 4-byte LDWEIGHTS (224ns)
+ 54ns handoff beats the 213ns moving stream -> 280ns/MM. fp16 halves
LDW bytes and enables compiler-automatic FWL -> ~216ns/MM (the N=512
stream floor). Mixed dtype is illegal (walrus verifier). fp16 10-bit
mantissa keeps absmax rel err ~2e-4 (gate 2e-2).

Token-parallel (vs 4x2 token x dout) halves the per-core PT = A@x_c^T
overhead (PT moving-cycles scale with tokens/core): one [8,512] PSUM
bank, one PT matmul per k-chunk. Structure per core: 8 o-blocks of 512
douts; per block 4 token-tile accumulators (PSUM tags acc0-3 x bufs=2,
PT rides acc3's second buf). The last o-block runs as two 2-token
passes so the final drain is 2 tiles, not 8; y-DMAs alternate
scalar/sync rings so evac never throttles on one ring.
"""
import sys

sys.path.insert(0, "/opt/trn_rl_repo")

import numpy as np

T_SH = 8                    # token shards (pure data-parallel)
T, D, O = 4096, 4096, 4096  # flattened tokens, d_in, d_out
TC, OC = T // T_SH, O       # 512 tokens per core, full 4096 douts
KC = D // 128               # 32 contraction chunks
NB = OC // 512              # 8 o-blocks of 512
TT = TC // 128              # 4 token tiles per core
WG = 4                      # base^T chunks per streaming DMA

_cache = {}


def _build():
    import concourse.mybir as mybir
    import concourse.tile as tile
    from concourse import bacc

    f32 = mybir.dt.float32
    fp16 = mybir.dt.float16

    nc = bacc.Bacc("TRN2", target_bir_lowering=False, debug=False,
                   num_devices=8)

    xt_d = nc.dram_tensor("xt", [D, TC], fp16, kind="ExternalInput").ap()
    wt_d = nc.dram_tensor("wt", [D, OC], fp16, kind="ExternalInput").ap()
    # at/bb/ptw are zero-PADDED to full 128x128 stationary / 128-partition
    # moving shapes: a matmul whose tile_size differs from its neighbours
    # (PT's 8-col output, the K=9 close) costs ~95ns extra on entry AND
    # exit (PE array tile reconfiguration) — 2x per k-chunk in o-block 0.
    # Zero rows/columns contribute nothing, so the math is unchanged.
    at_d = nc.dram_tensor("at", [128, KC, 128], fp16,
                          kind="ExternalInput").ap()
    # rows 0-7: 2*B^T, row 8: bias, rows 9-127: zero
    bb_d = nc.dram_tensor("bb", [128, OC], fp16, kind="ExternalInput").ap()
    ones_d = nc.dram_tensor("ones", [1, TC], fp16, kind="ExternalInput").ap()
    y_d = nc.dram_tensor("y", [TC, OC], f32, kind="ExternalOutput").ap()

    with tile.TileContext(nc) as tc:
        with (
            tc.tile_pool(name="res", bufs=1) as res,
            tc.tile_pool(name="wst", bufs=8) as wst,
            tc.tile_pool(name="evac", bufs=6) as evac,
            tc.tile_pool(name="psum", bufs=1, space="PSUM") as psum,
        ):
            # xt chunk 0 first so the first matmuls' data lands fast
            # (scalar = ACT HWDGE ring), then at for the PT matmuls
            xt = res.tile([128, KC, TC], fp16)
            xt_src = xt_d.rearrange("(c p) t -> c p t", p=128)
            nc.scalar.dma_start(xt[:, 0, 0:128], xt_src[0][:, 0:128])
            nc.scalar.dma_start(xt[:, 0, 128:TC], xt_src[0][:, 128:TC])
            # at is 1MB padded: stream the first 8 k-chunks now (PT needs
            # chunk k at ~10+1.1k us), the rest after xt7 so xt chunks
            # aren't pushed back behind a 1MB transfer
            # pace the 1MB padded-at stream in small slices behind the xt
            # chunks the PE consumes first: PT(k) runs ~1.1us after base(k),
            # so each at slice only has to beat its k-window, never an xt
            at = res.tile([128, KC, 128], fp16)
            nc.scalar.dma_start(at[:, 0:2, :], at_d[:, 0:2, :])
            at_sched = {5: (2, 8), 9: (8, 16), 13: (16, 24), 17: (24, KC)}
            for k in range(3, KC):
                nc.scalar.dma_start(xt[:, k, :], xt_src[k])
                if k in at_sched:
                    a0, a1 = at_sched[k]
                    nc.scalar.dma_start(at[:, a0:a1, :], at_d[:, a0:a1, :])

            # xt chunks 1-2 ride the sync ring (emitted inside o-block 0,
            # after W g0/g1, via the interleave hook): the scalar ring is
            # DMA-issue-rate-bound for the first ~6us and chunks 1-2 were
            # arriving right at their consumption deadline
            def xt_filler(g):
                if g in (0, 1):
                    k = g + 1
                    nc.sync.dma_start(xt[:, k, :], xt_src[k])
            # residents not needed until the first closes (~45us in)
            bb = res.tile([128, OC], fp16)
            nc.scalar.dma_start(bb[:], bb_d[:])
            # ptw rows 0-7: PT = A@x^T (device-computed), row 8: ones,
            # rows 9-127: zero (memset; DMA/copies overwrite rows 0-8)
            ptw = res.tile([128, TC], fp16)
            nc.vector.memset(ptw[:], 0.0)
            nc.scalar.dma_start(ptw[8:9, :], ones_d[:])

            wt_src = wt_d.rearrange("(c p) o -> p c o", p=128)
            ev_ring = [0]

            def close_and_evac(acc, t, osl, split_out=False):
                nc.tensor.matmul(acc[:], ptw[:, 128 * t:128 * (t + 1)],
                                 bb[:, osl], start=False, stop=True)
                ev = evac.tile([128, 512], f32, name=f"ev{t}", tag="ev")
                nc.vector.tensor_copy(ev[:], acc[:])
                tsl = slice(128 * t, 128 * (t + 1))
                if split_out:
                    # drain the final tile on both rings to shorten the tail
                    h = slice(osl.start, osl.start + 256)
                    h2 = slice(osl.start + 256, osl.stop)
                    nc.scalar.dma_start(y_d[tsl, h], ev[:, 0:256])
                    nc.sync.dma_start(y_d[tsl, h2], ev[:, 256:512])
                else:
                    ring = nc.scalar if ev_ring[0] % 2 == 0 else nc.sync
                    ev_ring[0] += 1
                    ring.dma_start(y_d[tsl, osl], ev[:])

            def o_block(ob, t_list, with_pt=False, interleave=None):
                osl = slice(512 * ob, 512 * (ob + 1))
                accs = {
                    t: psum.tile([128, 512], f32, name=f"acc{t}_{ob}",
                                 tag=f"acc{t}", bufs=2)
                    for t in t_list
                }
                # PT rides the second buf of acc3's tag (ob0 only): 4 accs +
                # PT = 5 banks live; later blocks rotate through the 2 bufs
                if with_pt:
                    pt = psum.tile([128, TC], f32, name="pt", tag="acc3",
                                   bufs=2)
                # first 4 chunks ride small tiles on their own tag so the
                # NEXT o-block's head data prefetches early
                groups = []
                for g, (c0, ng) in enumerate(
                        [(0, 2), (2, 2)] +
                        [(4 + WG * i, WG) for i in range((KC - 4) // WG)]):
                    wtile = wst.tile([128, ng, 512], fp16,
                                     name=f"wt{ob}_{g}",
                                     tag=("wt0" if ng == 2 else "wt"),
                                     bufs=(3 if ng == 2 else None))
                    if ob == 0 and with_pt and g == 0:
                        # split the very first weight tile for a fast start
                        for j in range(ng):
                            nc.sync.dma_start(
                                wtile[:, j, :], wt_src[:, c0 + j, osl])
                    else:
                        nc.sync.dma_start(
                            wtile[:], wt_src[:, c0:c0 + ng, osl])
                    groups.append((c0, ng, wtile))
                    if interleave is not None:
                        interleave(g)
                for c0, ng, wtile in groups:
                    for j in range(ng):
                        k = c0 + j
                        for t in t_list:
                            nc.tensor.matmul(
                                accs[t][:],
                                xt[:, k, 128 * t:128 * (t + 1)],
                                wtile[:, j, :],
                                start=(k == 0), stop=False)
                        # PT after the base MMs: a marginally late at-chunk
                        # can't head-of-line block the whole k-iteration
                        if with_pt:
                            nc.tensor.matmul(pt[:], at[:, k, :], xt[:, k, :],
                                             start=(k == 0), stop=(k == KC - 1))
                if with_pt:
                    nc.vector.tensor_copy(ptw[0:8, :], pt[0:8, :])
                for t in t_list:
                    close_and_evac(accs[t], t, osl)

            o_block(0, list(range(TT)), with_pt=True, interleave=xt_filler)
            for ob in range(1, NB - 2):
                o_block(ob, list(range(TT)))

            # Last o-block: token-outer over a RESIDENT copy of its weights
            # (tag wlast, prefetched interleaved with block NB-2's stream),
            # so its 4 closes stagger ~7us apart and the final drain is a
            # single split tile instead of a bunched pair.
            osl_last = slice(512 * (NB - 1), 512 * NB)
            wl_groups = []

            def prefetch_wlast(g):
                c0, ng = ([(0, 2), (2, 2)] +
                          [(4 + WG * i, WG) for i in range((KC - 4) // WG)])[g]
                wtile = wst.tile([128, ng, 512], fp16, name=f"wl_{g}",
                                 tag="wlast", bufs=9)
                nc.sync.dma_start(wtile[:], wt_src[:, c0:c0 + ng, osl_last])
                wl_groups.append((c0, ng, wtile))

            o_block(NB - 2, list(range(TT)), interleave=prefetch_wlast)

            for t in range(TT):
                acc = psum.tile([128, 512], f32, name=f"acc{t}_last",
                                tag=f"acc{t}", bufs=2)
                for c0, ng, wtile in wl_groups:
                    for j in range(ng):
                        k = c0 + j
                        nc.tensor.matmul(
                            acc[:], xt[:, k, 128 * t:128 * (t + 1)],
                            wtile[:, j, :], start=(k == 0), stop=False)
                close_and_evac(acc, t, osl_last, split_out=(t == TT - 1))

    nc.compile()
    return nc


def _get_nc():
    if "nc" not in _cache:
        _cache["nc"] = _build()
    return _cache["nc"]


def kernel(x, base_weight, lora_A, lora_B, bias, _trace=False, _trace_kwargs=None):
    from concourse.bass_utils import run_bass_kernel_spmd

    nc = _get_nc()

    x_flat = np.ascontiguousarray(x, dtype=np.float32).reshape(T, D)
    xT = x_flat.T
    wt = np.ascontiguousarray(base_weight.T).astype(np.float16)
    at_core = np.ascontiguousarray(
        lora_A.T, dtype=np.float32).reshape(KC, 128, 8).transpose(1, 0, 2)
    at = np.zeros((128, KC, 128), dtype=np.float16)
    at[:, :, 0:8] = at_core.astype(np.float16)
    bb = np.zeros((128, O), dtype=np.float16)
    bb[0:8, :] = (2.0 * lora_B.T).astype(np.float16)
    bb[8, :] = bias.astype(np.float16)
    ones = np.ones((1, TC), dtype=np.float16)

    xt_shards = [np.ascontiguousarray(
        xT[:, TC * i:TC * (i + 1)]).astype(np.float16) for i in range(T_SH)]

    in_maps = []
    for c in range(8):
        in_maps.append({
            "xt": xt_shards[c],
            "wt": wt,
            "at": at,
            "bb": bb,
            "ones": ones,
        })

    res = run_bass_kernel_spmd(nc, in_maps, list(range(8)),
                               trace=_trace, **(_trace_kwargs or {}))

    y = np.empty((T, O), dtype=np.float32)
    for c in range(8):
        y[TC * c:TC * (c + 1), :] = res.results[c]["y"]
    out = y.reshape(x.shape[0], x.shape[1], O)
    if _trace:
        return out, res
    return out
